# revision 55
# baseline (speedup 1.0000x reference)
"""Trainium2 Bass kernel for nn_Agent_50500225466537 (retrieval_knn GCN agent).

Strategy (8-core SPMD, 1D row-shard of the N=8192 node dim, ZERO collectives,
ZERO GpSimd ops — pure HWDGE + PE/DVE/ACT):
  - Host prep computes everything that is O(N^2) or smaller and data-layout
    shaped: the GCN degrees d = colsum(A+I), the scaled layer-1 features
    Md = (X @ W1) * 64/d  (fp8, DoubleRow pair-tiled), and the pre-tiled
    (A+I)^T shards (partition-major int8->fp8 via LUT) so every DMA slice
    is one contiguous 8KB read per partition.
  - Device, per graph (y then x), each core fully independent, does the
    dominant O(N^2 D) work:
      S^T = Md^T @ AhT                fp8 DoubleRow matmuls consuming the
                                      1MB at-slices as they stream
      h^T = sigmoid(S^T / (64 d_i) + b1) -> bf16 -> DMA out
  - A self-paced junk-matmul warmup keeps the PE HAM clock gate busy
    through the DMA lead-in, and the ACT sigmoid table is preloaded once
    (nothing evicts it — ACT only runs sigmoids).
  - Host tail: u = h @ W2 per core, then layer 2 collapses to matvecs
    because W2 is (256, 1): G = sigmoid(((A+I) @ (u/d))/d + b2).  G_y via
    one BLAS matvec, g_x via one row-dot, then the cosine top-11 + softmax
    exactly as the reference.
"""
import os
import sys

for _p in ("/opt/trn_rl_repo", "/root/.axon_site/_ro/trn_rl_repo"):
    if os.path.isdir(_p) and _p not in sys.path:
        sys.path.insert(0, _p)

import numpy as np

import concourse.bacc as bacc
from concourse import bass_utils, mybir, tile

N = 8192
NCORES = 8
R = N // NCORES          # rows per core: 1024
PB = 128                 # partition block
KB = N // PB             # 64 k-blocks
KB2 = KB // 2            # 32 k-block pairs (fp8 DoubleRow)
D = 256                  # feature dim (= hidden dim)
CHUNK = 4                # kb2-tiles per at DMA slice (1MB per slice)
NCHUNK = KB2 // CHUNK    # 8 slices per graph shard
MCH = 4                  # md DMA chunks per graph (8 kb2-pairs each)
MW = KB2 // MCH          # 8
EPS = 1e-8
K_OPP = 11
MDS = 64.0               # fp8 scale for Md (power of two, exact)
NWARM = 8                # junk matmuls to pre-warm the PE clock

F32 = mybir.dt.float32
BF16 = mybir.dt.bfloat16
FP8 = mybir.dt.float8e4
AF = mybir.ActivationFunctionType
DR = mybir.MatmulPerfMode.DoubleRow


class _G:
    """Per-graph emission state."""
    pass


def _at_slice_dma(nc, P, g, t0, nk=CHUNK):
    """One streaming slice (nk kb2-tiles) of this graph's AhT shard."""
    c = P.at.tile([PB, nk, 2, R], FP8, tag="at", name="at")
    nc.sync.dma_start(c[:], g.ahT[:, t0:t0 + nk])
    for j in range(nk):
        g.atmap[t0 + j] = (c, j)


def _md_chunk_dma(nc, P, g, q0, nq=MW):
    """One chunk (nq kb2-pairs) of host-precomputed scaled Md."""
    m = P.mdc.tile([PB, nq, 2, D], FP8, tag="mdc", name="mdc")
    nc.sync.dma_start(m[:], g.mdq[:, q0:q0 + nq])
    for j in range(nq):
        g.mdmap[q0 + j] = (m, j)


def _bigmm_block(nc, P, g, blk):
    """4 kb2 steps of S^T = Md^T @ AhT accumulation (16 matmuls).
    psS[nh] is a [PB, 1024] 2-bank tile; each (nh, ih) quadrant
    accumulates within a single bank."""
    for kb2 in range(blk * CHUNK, (blk + 1) * CHUNK):
        at, j = g.atmap[kb2]
        md, j2 = g.mdmap[kb2]
        for nh in range(2):
            for ih in range(2):
                nc.tensor.matmul(
                    g.psS[nh][:, ih * 512:(ih + 1) * 512],
                    md[:, j2, :, nh * PB:(nh + 1) * PB],
                    at[:, j, :, ih * 512:(ih + 1) * 512],
                    start=(kb2 == 0), stop=(kb2 == KB2 - 1), perf_mode=DR)


def _stage_graph(nc, P, g):
    """All 8 streaming bigmm blocks for one graph."""
    g.psS = [P.ps_s.tile([PB, 2 * 512], F32, tag="psS", name="psS")
             for _ in range(2)]
    for blk in range(NCHUNK):
        _bigmm_block(nc, P, g, blk)


def _stage_epi(nc, P, g):
    """h^T = sigmoid(S^T * rb + b1) -> bf16 -> DMA out.  The u = h @ W2
    reduction happens on host from hT."""
    hT = P.hT.tile([PB, 2, R], BF16, tag=f"hT{g.tag}", name="hT")
    for nh in range(2):
        p = g.psS[nh]
        nc.vector.tensor_mul(p[:], p[:], g.rb)
        nc.scalar.activation(hT[:, nh, :], p[:],
                             AF.Sigmoid, bias=P.b1_2[:, nh:nh + 1])
        nc.sync.dma_start(g.hT_out[:, nh, :], hT[:, nh, :])


_CACHED_NC = None


def _build_program():
    global _CACHED_NC
    if _CACHED_NC is not None:
        return _CACHED_NC
    nc = bacc.Bacc("TRN2", target_bir_lowering=False, debug=False,
                   enable_asserts=False, num_devices=NCORES)

    gy = _G()
    gx = _G()
    gy.tag, gx.tag = "y", "x"
    gx.ahT = nc.dram_tensor("ahT_x", [PB, KB2, 2, R], FP8,
                            kind="ExternalInput").ap()
    gy.ahT = nc.dram_tensor("ahT_y", [PB, KB2, 2, R], FP8,
                            kind="ExternalInput").ap()
    gx.mdq = nc.dram_tensor("mdq_x", [PB, KB2, 2, D], FP8,
                            kind="ExternalInput").ap()
    gy.mdq = nc.dram_tensor("mdq_y", [PB, KB2, 2, D], FP8,
                            kind="ExternalInput").ap()
    smf_in = nc.dram_tensor("smf32", [PB, 2], F32, kind="ExternalInput").ap()
    smb_in = nc.dram_tensor("smbf", [PB, 2 * R], BF16,
                            kind="ExternalInput").ap()

    gx.hT_out = nc.dram_tensor("hT_x", [PB, 2, R], BF16,
                               kind="ExternalOutput").ap()
    gy.hT_out = nc.dram_tensor("hT_y", [PB, 2, R], BF16,
                               kind="ExternalOutput").ap()

    with tile.TileContext(nc) as tc:
        P = _G()
        import contextlib
        with contextlib.ExitStack() as st:
            P.at = st.enter_context(tc.tile_pool(name="at", bufs=12))
            P.mdc = st.enter_context(tc.tile_pool(name="mdc", bufs=8))
            P.hT = st.enter_context(tc.tile_pool(name="hT", bufs=1))
            P.w = st.enter_context(tc.tile_pool(name="w", bufs=1))
            P.ps_s = st.enter_context(
                tc.tile_pool(name="ps_s", bufs=4, space="PSUM"))

            # ONE HWDGE queue (SP) in priority order: arrival order is
            # deterministic and matches PE consumption.
            smf = P.w.tile([PB, 2], F32, tag="smf", name="smf")
            nc.sync.dma_start(smf[:], smf_in)
            P.b1_2 = smf
            gy.atmap, gx.atmap, gy.mdmap, gx.mdmap = {}, {}, {}, {}
            # fine-grained head so the first bigmm matmuls start earlier:
            # 2-pair md piece + 1-tile at pieces at the very front
            _md_chunk_dma(nc, P, gy, 0, 2)
            _at_slice_dma(nc, P, gy, 0, 1)
            _at_slice_dma(nc, P, gy, 1, 1)
            _md_chunk_dma(nc, P, gy, 2, 2)
            _at_slice_dma(nc, P, gy, 2, 2)
            _md_chunk_dma(nc, P, gy, 4, 4)
            _at_slice_dma(nc, P, gy, 4)
            _md_chunk_dma(nc, P, gy, 8)
            _at_slice_dma(nc, P, gy, 8)
            _at_slice_dma(nc, P, gy, 12)
            _md_chunk_dma(nc, P, gy, 16)
            _at_slice_dma(nc, P, gy, 16)
            smb = P.w.tile([PB, 2 * R], BF16, tag="smb", name="smb")
            nc.sync.dma_start(smb[:], smb_in)
            gy.rb = smb[:, 0:R]
            gx.rb = smb[:, R:2 * R]
            _at_slice_dma(nc, P, gy, 20)
            _md_chunk_dma(nc, P, gy, 24)
            _at_slice_dma(nc, P, gy, 24)
            _at_slice_dma(nc, P, gy, 28)
            for c in range(MCH):
                _md_chunk_dma(nc, P, gx, c * MW)
                _at_slice_dma(nc, P, gx, c * MW)
                _at_slice_dma(nc, P, gx, c * MW + 4)

            # PE pre-warm: junk matmuls bridge the DMA lead-in (the last
            # two self-pace on the first real tiles); ACT sigmoid table
            # preloaded once — nothing ever evicts it.
            wu = P.w.tile([PB, 512], FP8, tag="wu", name="wu")
            nc.vector.memset(wu[:], 1.0)
            sigp = P.w.tile([1, 32], F32, tag="sigp", name="sigp")
            nc.scalar.activation(sigp[:], wu[0:1, 0:32], AF.Sigmoid)
            psw = P.ps_s.tile([PB, 2 * 512], F32, tag="psS", name="psw")
            for _ in range(NWARM):
                nc.tensor.matmul(psw[:, 0:512], wu[:, 0:128], wu[:],
                                 start=True, stop=True)
            nc.tensor.matmul(psw[:, 0:512], wu[:, 0:128],
                             gy.mdmap[0][0][:, 0, :, 0:D],
                             start=True, stop=True)
            nc.tensor.matmul(psw[:, 0:512], wu[:, 0:128],
                             gy.atmap[0][0][:, 0, :, 0:D],
                             start=True, stop=True)

            # PE order: y blocks, epi_y, x blocks, epi_x.  The psS ring
            # (4 bufs incl. the warmup tile) hands x fresh buffers so its
            # first block never waits on epi_y.
            _stage_graph(nc, P, gy)
            _stage_epi(nc, P, gy)
            _stage_graph(nc, P, gx)
            _stage_epi(nc, P, gx)

    nc.compile()
    _CACHED_NC = nc
    return nc


def _prep_in_maps(A_x, A_y, first_embeddings, second_embeddings, W1, b1, W2,
                  b2):
    import ml_dtypes

    # fp8 bit patterns for the exact small ints {0, 1, 2}
    lut = np.array([0.0, 1.0, 2.0], dtype=np.float32).astype(
        ml_dtypes.float8_e4m3fn).view(np.uint8)

    def prep_graph(A, X):
        d = (A.sum(axis=0, dtype=np.int64) + 1).astype(np.float32)
        A8 = A.astype(np.int8)
        A8[np.arange(N), np.arange(N)] += 1
        AT = np.ascontiguousarray(A8.T)  # AT[k, i] = (A+I)[i, k]
        shards = []
        for c in range(NCORES):
            blk = AT[:, c * R:(c + 1) * R].reshape(KB2, 2, PB, R)
            blk = np.ascontiguousarray(blk.transpose(2, 0, 1, 3))
            shards.append(lut[blk].view(ml_dtypes.float8_e4m3fn))
        # scaled layer-1 features, fp8 DoubleRow pair-tiled:
        # mdq[p, kb2, ko, h] = Md[kb2*256 + ko*128 + p, h] * 64/d_k
        Md = (X @ W1) * (np.float32(MDS) / d)[:, None]
        mdq = np.ascontiguousarray(
            Md.reshape(KB2, 2, PB, D).transpose(2, 0, 1, 3)).astype(
                ml_dtypes.float8_e4m3fn)
        return d, shards, mdq

    d_x, shx, mdq_x = prep_graph(A_x, first_embeddings)
    d_y, shy, mdq_y = prep_graph(A_y, second_embeddings)

    smf32 = np.ascontiguousarray(b1.reshape(2, PB).T)

    rb_x = (np.float32(1.0) / (np.float32(MDS) * d_x))
    rb_y = (np.float32(1.0) / (np.float32(MDS) * d_y))
    smbf_list = []
    for c in range(NCORES):
        s = np.empty((PB, 2 * R), dtype=np.float32)
        s[:, 0:R] = rb_y[c * R:(c + 1) * R][None, :]
        s[:, R:2 * R] = rb_x[c * R:(c + 1) * R][None, :]
        smbf_list.append(s.astype(ml_dtypes.bfloat16))

    in_maps = [
        dict(ahT_x=shx[c], ahT_y=shy[c], mdq_x=mdq_x, mdq_y=mdq_y,
             smf32=smf32, smbf=smbf_list[c])
        for c in range(NCORES)
    ]
    return in_maps, d_x, d_y


def _sigmoid(x):
    return 1.0 / (1.0 + np.exp(-x))


def kernel(A_x, A_y, first_embeddings, second_embeddings, W1, b1, W2, b2,
           W_h, W_f, W_p, bias_h, index_x, index_y):
    A_x = np.asarray(A_x)
    A_y = np.asarray(A_y)
    first_embeddings = np.asarray(first_embeddings, dtype=np.float32)
    second_embeddings = np.asarray(second_embeddings, dtype=np.float32)
    W1 = np.asarray(W1, dtype=np.float32)
    b1 = np.asarray(b1, dtype=np.float32)
    W2 = np.asarray(W2, dtype=np.float32)
    b2 = np.asarray(b2, dtype=np.float32)
    W_h = np.asarray(W_h, dtype=np.float32)
    W_f = np.asarray(W_f, dtype=np.float32)
    W_p = np.asarray(W_p, dtype=np.float32)
    bias_h = np.asarray(bias_h, dtype=np.float32)
    ix = int(index_x)
    iy = int(index_y)

    nc = _build_program()
    in_maps, d_x, d_y = _prep_in_maps(A_x, A_y, first_embeddings,
                                      second_embeddings, W1, b1, W2, b2)
    res = bass_utils.run_bass_kernel_spmd(nc, in_maps,
                                          core_ids=list(range(NCORES)))
    results = res.results

    W2_2 = W2[:, 0].reshape(2, PB).T.astype(np.float32)  # [PB, 2]

    def u_full(key):
        return np.concatenate([
            np.einsum("pki,pk->i",
                      np.asarray(results[c][key], dtype=np.float32), W2_2)
            for c in range(NCORES)])

    u_x = u_full("hT_x")
    u_y = u_full("hT_y")

    # ---- host tail (O(N^2) matvec + O(N) ops), fp32 like the reference ----
    row = A_x[ix].astype(np.float32)
    row[ix] += 1.0
    pre = np.float32(row @ (u_x / d_x)) / d_x[ix] + b2[0]
    g_x = _sigmoid(np.float32(pre))

    s = u_y / d_y
    w = A_y.astype(np.float32) @ s + s      # (A_y + I) @ s
    G_y_full = _sigmoid(w / d_y + b2[0]).astype(np.float32)
    g_y = G_y_full[iy]

    cat = np.array([[g_x], [g_y]], dtype=np.float32)        # (2, 1)
    h = _sigmoid(W_h @ cat + bias_h)                        # (1, 1)
    f = np.exp(g_x * W_f * g_y)                             # (1, 1)

    # cosine-similarity top-k over G_y (C = 1)
    num = G_y_full * g_y
    ng = np.maximum(np.abs(G_y_full), np.float32(EPS))
    nv = np.maximum(np.abs(g_y), np.float32(EPS))
    sims = num / (ng * nv)
    idx = np.argsort(-sims, kind="stable")[:K_OPP]
    opp = G_y_full[idx]
    f_oppo = np.float32(np.sum(np.exp(g_x * W_f[0, 0] * opp)))

    I_val = f / f_oppo                                      # (1, 1)
    z = W_p @ np.concatenate([h, I_val], axis=1)            # (1, 2)
    zs = z - z.max(axis=1, keepdims=True)
    ez = np.exp(zs)
    policy = ez / ez.sum(axis=1, keepdims=True)
    return policy.astype(np.float32)
